# revision 6
# baseline (speedup 1.0000x reference)
"""Trainium2 Bass kernel for nn_BaseTBLoRa (moe_routing).

out[b,s,:] = x[b,s,:] @ W_base.T + b_base + 2.0 * ((x[b,s,:] @ A_w[e_b].T) @ B_w[e_b].T)
with e_b = segment[b].  B=8, S=2048, D=1024, Do=1024, R=16, E=8.

Sharding: data-parallel over batch — core b handles batch b (B == n_cores).
No collectives; each core's output slice is gathered on the host.

The LoRA branch is merged into the base weight on the host (exact algebra):
    W_eff[e] = W_base + LORA_SCALE * B_w[e] @ A_w[e]          # [Do, D]
so each core runs one dense GEMM  y = x_b @ W_eff[e_b].T + b_base  in bf16
(fp8 DoubleRow was probed on HW: no 2x — each DR instr runs ~1 cyc/row —
so the 3-term fp8 split is strictly slower than bf16; bf16 it is).

HW probe facts this schedule is built on (dilation methodology, 8 cores):
  - PE sustained rate ~1.6GHz effective, NOT 2.4: 256 MMs of N=512 take
    80.7us (floor 54.6); zero per-instruction overhead (pehalf/pe256 fit),
    N=256 mix is 4.6% faster (77.0us), stationary swaps ~54ns apiece.
  - DMA aggregate ~183 GB/s for 1-2KB descriptors (model says 360),
    ~215 GB/s for >=2KB descriptors. In+out share the bottleneck.
  - For_i puts an all-engine barrier between iterations: each iteration
    is a full serial body (no cross-iteration overlap, p-state persists).

Schedule (per core, per iteration):
  - dt-major macro-chunks: per 512-column s-chunk, the 4 s-tiles accumulate
    in PARALLEL across the contraction dim (4 PSUM tiles = all 8 banks).
    This kills the st-major head-of-line stall where the first s-tile
    needed ALL of W (2MB @ 183GB/s ~ 11us) while later s-tiles sat queued.
    With dt-major, each arriving (x,W) dt-piece unlocks 4 s-tiles of work:
    PE consumption per dt-column (~2.6us) > arrival (~2.1us) => no stall
    after the first pair.
  - x is shipped in dt-pair-contiguous layout [NSC,128,NDT/2,2,SC] so each
    x DMA moves 2KB/partition descriptors (215 vs 183 GB/s).
  - N=256 matmuls (4 per s-tile per dt): probe-measured 4.6% faster PE
    stream than N=512. PSUM start/stop once per 2KB zero region (bank).
  - drain: DVE tensor_add fuses bias + fp32->bf16 cast; out DMA'd as bf16
    (host upconverts; rel err ~3.3e-3 vs the 2e-2 gate).
  - queue split: x on the SP HWDGE queue; W, bias and out on the Activation
    queue. Chunk-0's 3MB feed runs on two queues in parallel (outs only
    start after W is resident, so they don't contend). Sandwich A/B:
    v9 84.7us vs v6 (same minus queue-split/lead/tail trims) 88.4-99.2us.
  - lead/tail trims: the first x piece is split by pair-half and the first
    W piece by DO-half (first matmul fed in ~0.6us instead of ~2.4us); the
    last s-tile's drain runs in quarters to pipeline DVE -> out-DMA.

fp8 post-mortem (probed, rejected): DoubleRow with a pair-interleaved
moving operand delivers ~2.2x per MAC vs bf16 (283ns vs 647ns per dual-k
512-row block), but the 2e-2 gate forces a 3-term hi/lo split (3x MACs)
-> 108.6us total, slower than the bf16 stream. 2-term splits fail the
gate (rel err 0.027-0.029). Interleaved-stationary DR violates the ISA
('s3_lw_dual_fp8_restrictions'); SwInterleave mode measured 113.8us.
"""

import ml_dtypes
import numpy as np

import concourse.tile as tile
from concourse import bacc, mybir
from concourse.bass_utils import run_bass_kernel_spmd

LORA_SCALE = 32.0 / 16.0

B, S, D, DO, R = 8, 2048, 1024, 1024, 16
NDT = D // 128   # 8 contraction tiles
NST = S // 128   # 16 s-tiles
NSC = 4          # s macro-chunks
SC = S // NSC    # 512 s per macro-chunk
SUB = SC // 128  # 4 s-tiles per macro-chunk
N_HALF = 256     # matmul moving-dim tile (probe: N=256 beats N=512 by 4.6%)
N_CORES = 8

F32 = mybir.dt.float32
BF16 = mybir.dt.bfloat16

last_in_maps = None
last_results = None


def _build(loop_n=0):
    """loop_n > 0 wraps the body in a dynamic For_i (used only for dilation
    timing); the graded path uses loop_n=0 (straight-line program)."""
    import contextlib

    nc = bacc.Bacc("TRN2", target_bir_lowering=False, debug=False)

    x_d = nc.dram_tensor("x6", [NSC, 128, NDT // 2, 2, SC], BF16,
                         kind="ExternalInput")
    w_d = nc.dram_tensor("wt", [128, NDT, DO], BF16, kind="ExternalInput")
    bias_d = nc.dram_tensor("bias", [128, DO], BF16, kind="ExternalInput")
    out_d = nc.dram_tensor("out", [NST, 128, DO], BF16, kind="ExternalOutput")
    NH = DO // N_HALF

    with tile.TileContext(nc) as tc:
        with (
            tc.tile_pool(name="wpool", bufs=1) as wpool,
            tc.tile_pool(name="cpool", bufs=1) as cpool,
            tc.tile_pool(name="xpool", bufs=3) as xpool,
            tc.tile_pool(name="opool", bufs=4) as opool,
            tc.tile_pool(name="psy", bufs=1, space="PSUM") as psy,
        ):
            loop_cm = tc.For_i(0, loop_n, 1) if loop_n else contextlib.nullcontext()

            with loop_cm:
                bias_t = cpool.tile([128, DO], BF16)
                w_t = wpool.tile([128, NDT, DO], BF16)
                for sc in range(NSC):
                    x_t = xpool.tile([128, NDT // 2, 2, SC], BF16, name="x6t")
                    for dtp in range(NDT // 2):
                        # x on the SP queue (256KB per dt-pair, 2KB descs);
                        # W/bias on the Activation queue so chunk-0's feed
                        # runs on two queues in parallel
                        if sc == 0 and dtp == 0:
                            # split first pieces for a short lead-in
                            nc.sync.dma_start(x_t[:, 0, 0], x_d[0, :, 0, 0])
                            nc.sync.dma_start(x_t[:, 0, 1], x_d[0, :, 0, 1])
                            nc.scalar.dma_start(w_t[:, 0, 0:512],
                                                w_d[:, 0, 0:512])
                            nc.scalar.dma_start(w_t[:, 0, 512:1024],
                                                w_d[:, 0, 512:1024])
                            nc.scalar.dma_start(w_t[:, 1, :], w_d[:, 1, :])
                        else:
                            nc.sync.dma_start(x_t[:, dtp], x_d[sc, :, dtp])
                            if sc == 0:
                                nc.scalar.dma_start(w_t[:, 2 * dtp, :],
                                                    w_d[:, 2 * dtp, :])
                                nc.scalar.dma_start(w_t[:, 2 * dtp + 1, :],
                                                    w_d[:, 2 * dtp + 1, :])
                    if sc == 0:
                        nc.scalar.dma_start(bias_t[:], bias_d[:])

                    ps = [psy.tile([128, DO], F32, name=f"ps{i}")
                          for i in range(SUB)]
                    per_bank = max(1, 512 // N_HALF)
                    for dt in range(NDT):
                        for sub in range(SUB):
                            xt = x_t[:, dt // 2, dt % 2,
                                     sub * 128:(sub + 1) * 128]
                            for h in range(NH):
                                first = dt == 0 and h % per_bank == 0
                                last = (dt == NDT - 1
                                        and h % per_bank == per_bank - 1)
                                nc.tensor.matmul(
                                    ps[sub][:, h * N_HALF:(h + 1) * N_HALF],
                                    xt,
                                    w_t[:, dt, h * N_HALF:(h + 1) * N_HALF],
                                    start=first, stop=last)
                            if dt == NDT - 1:
                                st = sc * SUB + sub
                                o_t = opool.tile([128, DO], BF16)
                                # fused bias add + fp32->bf16 cast on DVE
                                if st == NST - 1:
                                    # pipeline the final drain in quarters
                                    for q in range(4):
                                        qs = slice(q * 256, (q + 1) * 256)
                                        nc.vector.tensor_add(
                                            o_t[:, qs], ps[sub][:, qs],
                                            bias_t[:, qs])
                                        nc.scalar.dma_start(
                                            out_d[st][:, qs], o_t[:, qs])
                                else:
                                    nc.vector.tensor_add(o_t[:], ps[sub][:],
                                                         bias_t[:])
                                    nc.scalar.dma_start(out_d[st], o_t[:])

    nc.compile()
    return nc


def _prep_core_inputs(x_b, wt5, bias_rep):
    x6 = np.ascontiguousarray(
        x_b.T.reshape(NDT, 128, NSC, SC).reshape(NDT // 2, 2, 128, NSC, SC)
        .transpose(3, 2, 0, 1, 4)).astype(ml_dtypes.bfloat16)
    return {"x6": x6, "wt": wt5, "bias": bias_rep}


def make_in_maps(x, seg, W, b, A_w, B_w):
    """Host-side prep for all 8 cores (used by kernel() and timing scripts)."""
    wt_by_expert = {}
    for e in set(int(v) for v in seg):
        w_eff = W + LORA_SCALE * (B_w[e] @ A_w[e])
        wt_by_expert[e] = np.ascontiguousarray(
            w_eff.T.reshape(NDT, 128, DO).transpose(1, 0, 2)
        ).astype(ml_dtypes.bfloat16)
    bias_rep = np.ascontiguousarray(
        np.broadcast_to(b, (128, DO))).astype(ml_dtypes.bfloat16)
    return [
        _prep_core_inputs(x[bb], wt_by_expert[int(seg[bb])], bias_rep)
        for bb in range(B)
    ]


def kernel(x, segment, W_base, b_base, A_w, B_w, _sim=False):
    global last_in_maps, last_results

    x = np.asarray(x, dtype=np.float32)
    W_base = np.asarray(W_base, dtype=np.float32)
    b_base = np.asarray(b_base, dtype=np.float32)
    A_w = np.asarray(A_w, dtype=np.float32)
    B_w = np.asarray(B_w, dtype=np.float32)
    seg = np.asarray(segment).astype(np.int64)

    in_maps = make_in_maps(x, seg, W_base, b_base, A_w, B_w)
    last_in_maps = in_maps

    nc = _build()

    if _sim:
        from concourse.bass_interp import CoreSim

        outs = []
        for b in range(B):
            sim = CoreSim(nc)
            for name, arr in in_maps[b].items():
                sim.tensor(name)[:] = arr
            sim.simulate()
            outs.append(
                np.asarray(sim.tensor("out")).astype(np.float32).reshape(S, DO))
        return np.stack(outs)

    res = run_bass_kernel_spmd(nc, in_maps, list(range(N_CORES)))
    last_results = res
    return np.stack([
        np.asarray(res.results[c]["out"]).astype(np.float32).reshape(S, DO)
        for c in range(N_CORES)
    ])


# revision 7
# speedup vs baseline: 1.1012x; 1.1012x over previous
"""Trainium2 Bass kernel for nn_BaseTBLoRa (moe_routing).

out[b,s,:] = x[b,s,:] @ W_base.T + b_base + 2.0 * ((x[b,s,:] @ A_w[e_b].T) @ B_w[e_b].T)
with e_b = segment[b].  B=8, S=2048, D=1024, Do=1024, R=16, E=8.

Sharding: data-parallel over batch — core b handles batch b (B == n_cores).
No collectives; each core's output slice is gathered on the host.

Key transformation vs the earlier on-device-LoRA version: the LoRA branch is
merged into the base weight on the host (standard LoRA weight-merge, exact
algebra):
    W_eff[e] = W_base + LORA_SCALE * B_w[e] @ A_w[e]          # [Do, D]
so each core runs a single dense GEMM
    y = x_b @ W_eff[e_b].T + b_base
with no on-device LoRA matmuls at all. This removes the h = x@A^T chunk
matmuls and the K-padded h@B^T matmuls (~20% of the PE stream time).
Same-session HW A/B vs the on-device-LoRA baseline: 105.8us -> 86.5us per
For_i iteration (-18%), matching the cost-model prediction (81.0 -> 66.6us).

Device program per core (bf16 matmuls, fp32 PSUM accumulation):
  preload: bias (bf16, replicated over partitions) via 1 DMA
  for each 512-wide s macro-chunk (4 of them):
    x chunk DMA'd per k-tile (8 x 128KB); W interleaved with chunk 0 (8 x 256KB)
    for each 128-row s-tile (4 per chunk):
      ps_y[s128, 0:512]    = sum_dt xT[dt, s128].T @ WT[dt, 0:512]    (8 MMs, N=512)
      ps_y[s128, 512:1024] = sum_dt xT[dt, s128].T @ WT[dt, 512:]     (8 MMs, N=512)
      o = DVE tensor_add(ps_y, bias_rep)    # fused bias add + bf16 cast
      DMA o to DRAM (256KB)

Measured/modeled design points:
  - 256 MMs of N=512 bf16: 512 cyc @ 2.4 GHz = 213 ns/MM back-to-back;
    54.6 us PE-stream floor (cost-model total 66.6 us incl. DMA lead-in and
    p-state ramp; HW dilation ~85-100 us/iter depending on thermal state).
  - Fine-grained DMA beats batched 1MB DMAs on HW AND in the cost model
    (91.5 vs 86.1 us HW same-session): per-k-tile pieces keep the first MM
    group fed progressively; batching delays the pipeline more than the
    per-descriptor fixed cost saves.
  - bias add rides the DVE during the PSUM->SBUF drain (free: DVE has 3x
    slack vs PE); bias is bf16, and its DMA is issued AFTER chunk 0's x/W
    DMAs (it gates only the first drain, ~12us in; issuing it first cost
    ~1us of x/W lead-in). With that, 4 PSUM bufs beat 3 (model 64.1us total
    vs 66.6us; the delta is sub-noise in dilation but real for one-shot).
  - output DMA'd as bf16 (halves store traffic); host upconverts to fp32.
    Adds <=2^-9*|y| rounding; total rel err 3.3e-3 vs the 2e-2 gate.
"""

import ml_dtypes
import numpy as np

import concourse.tile as tile
from concourse import bacc, mybir
from concourse.bass_utils import run_bass_kernel_spmd

LORA_SCALE = 32.0 / 16.0

B, S, D, DO, R = 8, 2048, 1024, 1024, 16
NDT = D // 128   # 8 contraction tiles
NST = S // 128   # 16 s-tiles
NSC = 4          # s macro-chunks
SC = S // NSC    # 512 s per macro-chunk
SUB = SC // 128  # 4 s-tiles per macro-chunk
N_CORES = 8

F32 = mybir.dt.float32
BF16 = mybir.dt.bfloat16

last_in_maps = None
last_results = None


def _build(loop_n=0):
    """loop_n > 0 wraps the body in a dynamic For_i (used only for dilation
    timing); the graded path uses loop_n=0 (straight-line program)."""
    import contextlib

    nc = bacc.Bacc("TRN2", target_bir_lowering=False, debug=False)

    x_d = nc.dram_tensor("x5", [NSC, 128, NDT, SC], BF16, kind="ExternalInput")
    w_d = nc.dram_tensor("wt", [128, NDT, DO], BF16, kind="ExternalInput")
    bias_d = nc.dram_tensor("bias", [128, DO], BF16, kind="ExternalInput")
    out_d = nc.dram_tensor("out", [NST, 128, DO], BF16, kind="ExternalOutput")

    with tile.TileContext(nc) as tc:
        with (
            tc.tile_pool(name="wpool", bufs=1) as wpool,
            tc.tile_pool(name="cpool", bufs=1) as cpool,
            tc.tile_pool(name="xpool", bufs=3) as xpool,
            tc.tile_pool(name="opool", bufs=4) as opool,
            tc.tile_pool(name="psy", bufs=4, space="PSUM") as psy,
        ):
            loop_cm = tc.For_i(0, loop_n, 1) if loop_n else contextlib.nullcontext()

            def preload():
                # bias DMA is issued LATER (after chunk 0's x/W DMAs): it is
                # only needed by the first DVE drain ~12us in, and putting it
                # first on the ring delayed x/W arrival by ~1us (model -1.9us).
                bias_t = cpool.tile([128, DO], BF16)
                w_t = wpool.tile([128, NDT, DO], BF16)
                return bias_t, w_t

            # A For_i body may not touch tiles allocated outside the loop, so
            # in timing mode the preload moves inside (slightly conservative).
            if not loop_n:
                bias_t, w_t = preload()
            with loop_cm:
                if loop_n:
                    bias_t, w_t = preload()
                for sc in range(NSC):
                    x_t = xpool.tile([128, NDT, SC], BF16)
                    for dt in range(NDT):
                        nc.sync.dma_start(x_t[:, dt, :], x_d[sc, :, dt, :])
                        if sc == 0:
                            # interleave the W preload with the first x chunk
                            nc.sync.dma_start(w_t[:, dt, :], w_d[:, dt, :])
                    if sc == 0:
                        nc.sync.dma_start(bias_t[:], bias_d[:])

                    for sub in range(SUB):
                        st = sc * SUB + sub
                        ps_y = psy.tile([128, DO], F32)
                        for dt in range(NDT):
                            xt = x_t[:, dt, sub * 128:(sub + 1) * 128]
                            first = dt == 0
                            last = dt == NDT - 1
                            nc.tensor.matmul(
                                ps_y[:, 0:512], xt, w_t[:, dt, 0:512],
                                start=first, stop=last,
                            )
                            nc.tensor.matmul(
                                ps_y[:, 512:1024], xt, w_t[:, dt, 512:1024],
                                start=first, stop=last,
                            )

                        o_t = opool.tile([128, DO], BF16)
                        # fused bias add + fp32->bf16 cast during PSUM drain
                        nc.vector.tensor_add(o_t[:], ps_y[:], bias_t[:])
                        nc.sync.dma_start(out_d[st], o_t[:])

    nc.compile()
    return nc


def _prep_core_inputs(x_b, wt5, bias_rep):
    xT = x_b.T.reshape(NDT, 128, NSC, SC).transpose(2, 1, 0, 3).astype(
        ml_dtypes.bfloat16
    )
    return {"x5": xT, "wt": wt5, "bias": bias_rep}


def make_in_maps(x, seg, W, b, A_w, B_w):
    """Host-side prep for all 8 cores (used by kernel() and timing scripts)."""
    wt_by_expert = {}
    for e in set(int(v) for v in seg):
        w_eff = W + LORA_SCALE * (B_w[e] @ A_w[e])
        wt_by_expert[e] = np.ascontiguousarray(
            w_eff.T.reshape(NDT, 128, DO).transpose(1, 0, 2)
        ).astype(ml_dtypes.bfloat16)
    bias_rep = np.ascontiguousarray(
        np.broadcast_to(b, (128, DO))).astype(ml_dtypes.bfloat16)
    return [
        _prep_core_inputs(x[bb], wt_by_expert[int(seg[bb])], bias_rep)
        for bb in range(B)
    ]


def kernel(x, segment, W_base, b_base, A_w, B_w, _sim=False):
    global last_in_maps, last_results

    x = np.asarray(x, dtype=np.float32)
    W_base = np.asarray(W_base, dtype=np.float32)
    b_base = np.asarray(b_base, dtype=np.float32)
    A_w = np.asarray(A_w, dtype=np.float32)
    B_w = np.asarray(B_w, dtype=np.float32)
    seg = np.asarray(segment).astype(np.int64)

    in_maps = make_in_maps(x, seg, W_base, b_base, A_w, B_w)
    last_in_maps = in_maps

    nc = _build()

    if _sim:
        from concourse.bass_interp import CoreSim

        outs = []
        for b in range(B):
            sim = CoreSim(nc)
            for name, arr in in_maps[b].items():
                sim.tensor(name)[:] = arr
            sim.simulate()
            outs.append(
                np.asarray(sim.tensor("out")).astype(np.float32).reshape(S, DO))
        return np.stack(outs)

    res = run_bass_kernel_spmd(nc, in_maps, list(range(N_CORES)))
    last_results = res
    return np.stack([
        np.asarray(res.results[c]["out"]).astype(np.float32).reshape(S, DO)
        for c in range(N_CORES)
    ])

